# revision 30
# baseline (speedup 1.0000x reference)
"""Trainium2 Bass kernel: ViT attention block with 2D RoPE (croco-style).

Full inputs -> full outputs. Sharding: data-parallel over batch, one batch
element per NeuronCore (B=8 across 8 cores), no collectives.

v3: software-pipelined head pairs.
  - Host-side transposes (x^T, w^T) -> straight DMAs, no xbar transposes.
  - Host-side cls-token qkv projection (0.02% of FLOPs) -> patch-aligned
    tiling everywhere; no 1-column matmul leftovers.
  - Per head pair j: qkv -> fix/pall rows -> rope -> attention, with the
    next pair's matmuls interleaved into pair j's attention loop as PE
    filler so the tensor engine never idles (keeps max p-state).
  - exp on ACT engine only; copies on Pool (gpsimd); rope/normalize on DVE.
  - One reciprocal_approx_fast per pair on [2,1025] (vs 12x [1,1024] full
    Newton reciprocal).
  - Projection computed transposed (out^T = Wp @ o^T) with free-dim-512
    blocks; host re-transposes the [768,1024] result.
"""

import numpy as np
import ml_dtypes

import concourse.bass as bass
import concourse.mybir as mybir
import concourse.tile as tile
from concourse import bacc
from concourse.bass_utils import run_bass_kernel_spmd

F32 = mybir.dt.float32
BF16 = mybir.dt.bfloat16
EXP = mybir.ActivationFunctionType.Exp
IDN = mybir.ActivationFunctionType.Identity
CPY = mybir.ActivationFunctionType.Copy

DIM = 768
H = 12
HD = 64
N = 1025
NP = 1024    # patch tokens
NC = 8
SCALE = HD ** -0.5

_CACHE = {}


def _build_body(tc):
    nc = tc.nc
    import contextlib, os as _os
    ctx = contextlib.ExitStack()
    pool_eng = nc.vector if _os.environ.get("K_NOPOOL", "0") == "1" else nc.gpsimd
    safe_recip = _os.environ.get("K_SAFERECIP", "0") == "1"

    def _recip(out_ap, in_ap):
        if safe_recip:
            nc.vector.reciprocal(out_ap, in_ap)
        else:
            nc.vector.reciprocal_approx_fast(out=out_ap, in_=in_ap)

    # ---- DRAM tensors (all host-prepped layouts) ----
    xt_d = nc.dram_tensor("xt", [DIM, NP], BF16, kind="ExternalInput")      # x[1:].T
    wqk_d = nc.dram_tensor("wqk", [DIM, 1536], BF16, kind="ExternalInput")  # w_qkv[0:1536].T
    wv_d = nc.dram_tensor("wv", [DIM, 792], BF16, kind="ExternalInput")     # w_v.T head-strided
    wp_d = nc.dram_tensor("wp", [DIM, DIM], BF16, kind="ExternalInput")     # w_proj.T
    ct_d = nc.dram_tensor("ct", [128, NP], BF16, kind="ExternalInput")
    st_d = nc.dram_tensor("st", [128, NP], BF16, kind="ExternalInput")
    pm_d = nc.dram_tensor("pm", [128, 128], BF16, kind="ExternalInput")
    id_d = nc.dram_tensor("ident", [128, 128], F32, kind="ExternalInput")
    idb_d = nc.dram_tensor("identb", [128, 128], BF16, kind="ExternalInput")
    kcb_d = nc.dram_tensor("kcb", [128, 12], BF16, kind="ExternalInput")    # k_cls packed
    qcb_d = nc.dram_tensor("qcb", [128, 12], BF16, kind="ExternalInput")    # q_cls packed
    vcm_d = nc.dram_tensor("vcm", [2, 792], BF16, kind="ExternalInput")     # v_cls parity
    vcr_d = nc.dram_tensor("vcr", [1, 792], BF16, kind="ExternalInput")     # v_cls row
    sel_d = nc.dram_tensor("sel2", [2, 128], BF16, kind="ExternalInput")
    selab_d = nc.dram_tensor("selab", [2, 97, 128], BF16, kind="ExternalInput")
    on_d = nc.dram_tensor("ones12", [128, 12], BF16, kind="ExternalInput")
    pcc_d = nc.dram_tensor("pcc", [12, 1], BF16, kind="ExternalInput")      # exp(s*qc.kc)
    bia_d = nc.dram_tensor("biac", [128, 6], F32, kind="ExternalInput")     # b_proj chunks
    bpr_d = nc.dram_tensor("bprow", [1, DIM], F32, kind="ExternalInput")    # b_proj row
    outT_d = nc.dram_tensor("outT", [DIM, NP], F32, kind="ExternalOutput")
    outc_d = nc.dram_tensor("outc", [1, DIM], F32, kind="ExternalOutput")

    # ---- persistent SBUF ----
    const = ctx.enter_context(tc.tile_pool(name="const", bufs=1))
    pmt = const.tile([128, 128], BF16, name="pmt")
    ctt = const.tile([128, NP], BF16, name="ctt")
    stt = const.tile([128, NP], BF16, name="stt")
    identt = const.tile([128, 128], F32, name="identt")
    identb = const.tile([128, 128], BF16, name="identb")
    kcbt = const.tile([128, 12], BF16, name="kcbt")
    qcbt = const.tile([128, 12], BF16, name="qcbt")
    vcmt = const.tile([2, 792], BF16, name="vcmt")
    vcrt = const.tile([1, 792], BF16, name="vcrt")
    selt = const.tile([2, 128], BF16, name="selt")
    selab0 = const.tile([97, 128], BF16, name="selab0")
    selab1 = const.tile([97, 128], BF16, name="selab1")
    biat = const.tile([128, 6], F32, name="biat")
    bprt = const.tile([1, DIM], F32, name="bprt")

    main = ctx.enter_context(tc.tile_pool(name="main", bufs=1))
    xT = [main.tile([128, NP], BF16, name=f"xT{j}") for j in range(6)]
    wqkT = [main.tile([128, 1536], BF16, name=f"wqkT{j}") for j in range(6)]
    wvT = [main.tile([128, 792], BF16, name=f"wvT{j}") for j in range(6)]
    wpT = [main.tile([128, DIM], BF16, name=f"wpT{j}") for j in range(6)]
    qT = [main.tile([128, NP], BF16, name=f"qT{j}") for j in range(6)]
    kT = [main.tile([128, NP], BF16, name=f"kT{j}") for j in range(6)]
    vA = [main.tile([128, 792], BF16, name=f"vA{t}") for t in range(8)]
    oT = [main.tile([128, N], BF16, name=f"oT{j}") for j in range(6)]
    # per-pair row data lives on partitions 0:2, column-concatenated by pair
    # (PE matmul operands require base partition in {0,32,64})
    fixE = main.tile([2, 6 * NP], BF16, name="fixE")
    pall = main.tile([2, 6 * N], BF16, name="pall")
    # denominators: head A on partition 0, head B on partition 32 (engine
    # APs need 32-aligned partition bases); pair j at cols j*1024.
    # rows {0,64}: head A qh0/qh1; rows {32,96}: head B qh0/qh1 (32-aligned)
    den2 = main.tile([97, 6 * 512], F32, name="den2")
    den2b = main.tile([97, 6 * 512], BF16, name="den2b")
    dclsr = main.tile([2, 6], F32, name="dclsr")
    dclsr2 = main.tile([2, 6], F32, name="dclsr2")
    recs = main.tile([97, 512], F32, name="recs")
    ctsb = main.tile([12, 792], F32, name="ctsb")
    pallGb = main.tile([12, N], BF16, name="pallGb")
    pallG = main.tile([12, N], F32, name="pallG")
    pT = [main.tile([128, 12], BF16, name=f"pT{t}") for t in range(8)]
    pclsTs = main.tile([1, 12], BF16, name="pclsTs")
    clsrow = main.tile([12, 64], F32, name="clsrow")

    etp = ctx.enter_context(tc.tile_pool(name="etp", bufs=1))
    rtp = ctx.enter_context(tc.tile_pool(name="rtp", bufs=1))
    osp = ctx.enter_context(tc.tile_pool(name="osp", bufs=1))

    # PSUM pools: fill 2x[128,512] (2 banks) + eps 2x[128,1024] (4 banks)
    # + oacc 1x[66,1024] (2 banks) = 8 banks.
    psF = ctx.enter_context(tc.tile_pool(name="psF", bufs=1, space="PSUM"))
    psE = ctx.enter_context(tc.tile_pool(name="psE", bufs=1, space="PSUM"))
    psO = ctx.enter_context(tc.tile_pool(name="psO", bufs=1, space="PSUM"))

    def fill():
        return psF.tile([128, 512], F32, name="fl", tag="fl", bufs=2)

    # ---- DMAs, ordered so early compute unblocks first ----
    nc.sync.dma_start(pmt[:], pm_d[:])
    nc.sync.dma_start(ctt[:], ct_d[:])
    nc.sync.dma_start(stt[:], st_d[:])
    for j in range(6):
        nc.sync.dma_start(xT[j][:], xt_d[j * 128:(j + 1) * 128, :])
    for j in range(6):  # q/k cols for pairs 0,1 first
        nc.sync.dma_start(wqkT[j][:, 0:256], wqk_d[j * 128:(j + 1) * 128, 0:256])
        nc.sync.dma_start(wqkT[j][:, 768:1024], wqk_d[j * 128:(j + 1) * 128, 768:1024])
    nc.sync.dma_start(identt[:], id_d[:])
    nc.sync.dma_start(identb[:], idb_d[:])
    nc.sync.dma_start(kcbt[:], kcb_d[:])
    nc.sync.dma_start(qcbt[:], qcb_d[:])
    nc.sync.dma_start(vcmt[:], vcm_d[:])
    nc.sync.dma_start(vcrt[:], vcr_d[:])
    nc.sync.dma_start(selt[:], sel_d[:])
    nc.sync.dma_start(selab0[:], selab_d[0])
    nc.sync.dma_start(selab1[:], selab_d[1])
    nc.sync.dma_start(biat[:], bia_d[:])
    nc.sync.dma_start(bprt[:], bpr_d[:])
    for j in range(6):
        nc.sync.dma_start(pall[0:2, j * N + NP:j * N + N], pcc_d[2 * j:2 * j + 2, 0:1])
    for j in range(6):
        nc.sync.dma_start(wvT[j][:], wv_d[j * 128:(j + 1) * 128, :])
    for j in range(6):
        nc.sync.dma_start(wqkT[j][:, 256:768], wqk_d[j * 128:(j + 1) * 128, 256:768])
        nc.sync.dma_start(wqkT[j][:, 1024:1536], wqk_d[j * 128:(j + 1) * 128, 1024:1536])
    for j in range(6):
        nc.sync.dma_start(wpT[j][:], wp_d[j * 128:(j + 1) * 128, :])

    nc.vector.memset(den2b[0:97, :], 0.0)
    nc.vector.memset(den2[0:97, :], 0.0)

    # ---- emission helpers ----
    def warmup(n):
        for _ in range(n):
            ps = fill()
            nc.tensor.matmul(ps[:, 0:512], pmt[:], ctt[:, 0:512],
                             start=True, stop=True)

    def emit_v(t):
        # v rows for patch tokens t*128..t*128+127 -> vA[t] (head-strided
        # cols; cols 64,65 mod 66 are zero from the padded weight).
        for (c0, cw) in ((0, 512), (512, 280)):
            ps = fill()
            for kc in range(6):
                nc.tensor.matmul(ps[:, 0:cw],
                                 xT[kc][:, t * 128:(t + 1) * 128],
                                 wvT[kc][:, c0:c0 + cw],
                                 start=(kc == 0), stop=(kc == 5))
            nc.scalar.copy(vA[t][:, c0:c0 + cw], ps[:, 0:cw])
        nc.sync.dma_start(vA[t][:, 64::66], on_d[:, 0:12])

    def emit_qk_block(j, which, cb):
        # q^T (which=0) or k^T (which=1) tile j, patch column block cb.
        dst = qT[j] if which == 0 else kT[j]
        m0 = j * 128 if which == 0 else 768 + j * 128
        ps = fill()
        for kc in range(6):
            nc.tensor.matmul(ps[:, 0:512],
                             wqkT[kc][:, m0:m0 + 128],
                             xT[kc][:, cb * 512:cb * 512 + 512],
                             start=(kc == 0), stop=(kc == 5))
        nc.vector.tensor_copy(dst[:, cb * 512:cb * 512 + 512], ps[:, 0:512])

    def emit_fp_block(j, which, cb):
        # fix rows (which=0: k_cls . q_patch -> fixE) or pall rows
        # (which=1: q_cls . k_patch -> pall), heads 2j,2j+1, col block cb.
        cstat = kcbt if which == 0 else qcbt
        rhs = qT[j] if which == 0 else kT[j]
        ps = fill()
        nc.tensor.matmul(ps[0:2, 0:512],
                         cstat[:, 2 * j:2 * j + 2],
                         rhs[:, cb * 512:cb * 512 + 512],
                         start=True, stop=True)
        if which == 0:
            nc.scalar.activation(fixE[0:2, j * NP + cb * 512:j * NP + cb * 512 + 512],
                                 ps[0:2, 0:512], EXP, scale=SCALE)
        else:
            nc.scalar.activation(pall[0:2, j * N + cb * 512:j * N + cb * 512 + 512],
                                 ps[0:2, 0:512], EXP, scale=SCALE)

    def emit_dcls(j):
        # row sums of exp'd pall rows (incl. cls-key col) -> dclsr col j
        nc.vector.tensor_reduce(out=dclsr[0:2, j:j + 1],
                                in_=pall[0:2, j * N:j * N + N],
                                op=mybir.AluOpType.add, axis=mybir.AxisListType.X)

    def emit_rope_block(tl, cb):
        ps = fill()
        nc.tensor.matmul(ps[:, 0:512], pmt[:], tl[:, cb * 512:cb * 512 + 512],
                         start=True, stop=True)
        tmp = rtp.tile([128, 512], BF16, name="rt", tag="rt", bufs=2)
        nc.vector.tensor_mul(tmp[:, :], ps[:, 0:512], stt[:, cb * 512:cb * 512 + 512])
        pool_eng.tensor_mul(tl[:, cb * 512:cb * 512 + 512],
                             tl[:, cb * 512:cb * 512 + 512],
                             ctt[:, cb * 512:cb * 512 + 512])
        nc.vector.tensor_add(tl[:, cb * 512:cb * 512 + 512],
                             tl[:, cb * 512:cb * 512 + 512], tmp[:, :])

    def emit_norm_block(j, cb):
        # rb = broadcast of 1/denom rows (heads 2j,2j+1, this qh) -> [128,512]
        db = j * 512
        sel = selab0 if cb == 0 else selab1
        ps = fill()
        nc.tensor.matmul(ps[:, 0:512], sel[0:97, :], den2b[0:97, db:db + 512],
                         start=True, stop=True)
        nc.vector.tensor_mul(oT[j][:, cb * 512:cb * 512 + 512],
                             oT[j][:, cb * 512:cb * 512 + 512], ps[:, 0:512])

    def emit_pall_gather(j):
        nc.sync.dma_start(pallGb[2 * j:2 * j + 2, 0:N], pall[0:2, j * N:j * N + N])

    def emit_pall_upcast():
        pool_eng.tensor_copy(pallG[0:12, 0:N], pallGb[0:12, 0:N])

    def emit_ptrans(t):
        ps = fill()
        nc.tensor.transpose(ps[0:128, 0:12], pallG[0:12, t * 128:(t + 1) * 128],
                            identt[0:12, 0:12])
        nc.vector.tensor_copy(pT[t][:, 0:12], ps[0:128, 0:12])

    def emit_pcls_trans():
        ps = fill()
        nc.tensor.transpose(ps[0:1, 0:12], pallG[0:12, NP:N], identt[0:12, 0:12])
        nc.vector.tensor_copy(pclsTs[0:1, 0:12], ps[0:1, 0:12])

    def emit_ctail(chunk):
        # cls-query attn@v: ctail[12, 792] = sum_t pT[t]^T@vA[t] + pcls^T@vcr
        c0, cw = (0, 512) if chunk == 0 else (512, 280)
        ps = fill()
        for t in range(8):
            nc.tensor.matmul(ps[0:12, 0:cw], pT[t][:, 0:12], vA[t][:, c0:c0 + cw],
                             start=(t == 0), stop=False, skip_group_check=True)
        nc.tensor.matmul(ps[0:12, 0:cw], pclsTs[0:1, 0:12], vcrt[0:1, c0:c0 + cw],
                         start=False, stop=True, skip_group_check=True)  # cls key
        nc.vector.tensor_copy(ctsb[0:12, c0:c0 + cw], ps[0:12, 0:cw])
        if chunk == 1:
            for h in range(12):
                nc.sync.dma_start(clsrow[h:h + 1, 0:64],
                                  ctsb[h:h + 1, h * 66:h * 66 + 64])

    # ---- stage B (attention) for pair j, with interleaved fillers ----
    def emit_B(j, fillers):
        hA, hB = 2 * j, 2 * j + 1
        its = [(qh, t) for qh in (0, 1) for t in range(8)]
        eps_t = {}
        et_t = {}

        def S(i):
            qh, t = its[i]
            ps = psE.tile([128, 1024], F32, name="eps", tag="eps", bufs=2)
            nc.tensor.matmul(ps[:, 0:512],
                             kT[j][0:64, t * 128:(t + 1) * 128],
                             qT[j][0:64, qh * 512:qh * 512 + 512],
                             start=True, stop=True)
            nc.tensor.matmul(ps[:, 512:1024],
                             kT[j][64:128, t * 128:(t + 1) * 128],
                             qT[j][64:128, qh * 512:qh * 512 + 512],
                             start=True, stop=True)
            et = etp.tile([128, 1024], BF16, name="et", tag="et", bufs=4)
            nc.scalar.activation(et[:, :], ps[:, :], EXP, scale=SCALE)
            eps_t[i] = ps
            et_t[i] = et

        import os as _os
        _ilv = _os.environ.get("K_ILV", "1") == "1"
        if not _ilv:
            for f in fillers:
                f()
            fillers = []
        S(0)
        S(1)
        oacc = None
        fi = 0
        nfill = len(fillers)
        for i, (qh, t) in enumerate(its):
            if i + 2 < 16:
                S(i + 2)
            # drain a fair share of fillers
            want = (i + 1) * nfill // 16
            while fi < want:
                fillers[fi]()
                fi += 1
            if t == 0:
                oacc = psO.tile([66, 1024], F32, name="oacc", tag="oacc", bufs=1)
            et = et_t.pop(i)
            nc.tensor.matmul(oacc[:, 0:512], vA[t][:, hA * 66:hA * 66 + 66],
                             et[:, 0:512],
                             start=(t == 0), stop=False, skip_group_check=True)
            nc.tensor.matmul(oacc[:, 512:1024], vA[t][:, hB * 66:hB * 66 + 66],
                             et[:, 512:1024],
                             start=(t == 0), stop=False, skip_group_check=True)
            if t == 7:
                # cls-key contribution (v_cls parity block x fixE rows)
                fb = j * NP + qh * 512
                nc.tensor.matmul(oacc[:, 0:512],
                                 vcmt[0:2, hA * 66:hA * 66 + 66],
                                 fixE[0:2, fb:fb + 512],
                                 start=False, stop=True, skip_group_check=True)
                nc.tensor.matmul(oacc[:, 512:1024],
                                 vcmt[0:2, hB * 66:hB * 66 + 66],
                                 fixE[0:2, fb:fb + 512],
                                 start=False, stop=True, skip_group_check=True)
                # flush: outputs + denominators
                db = j * 512
                ra = 0 if qh == 0 else 64
                rb_ = 32 if qh == 0 else 96
                nc.vector.tensor_copy(oT[j][0:64, qh * 512:qh * 512 + 512],
                                      oacc[0:64, 0:512])
                nc.vector.tensor_copy(oT[j][64:128, qh * 512:qh * 512 + 512],
                                      oacc[0:64, 512:1024])
                nc.vector.tensor_copy(den2[ra:ra + 1, db:db + 512],
                                      oacc[64:65, 0:512])
                nc.vector.tensor_copy(den2[rb_:rb_ + 1, db:db + 512],
                                      oacc[64:65, 512:1024])
        while fi < nfill:
            fillers[fi]()
            fi += 1
        # reciprocal of this pair's denominators
        jb = j * NP
        # one plain reciprocal across all 4 denominator rows, then casts
        jb2 = j * 512
        nc.vector.reciprocal(recs[0:97, 0:512], den2[0:97, jb2:jb2 + 512])
        for r in (0, 32, 64, 96):
            nc.vector.tensor_copy(den2b[r:r + 1, jb2:jb2 + 512],
                                  recs[r:r + 1, 0:512])

    # ---- full program ----
    stop_at = int(_os.environ.get("K_STOP", "99"))
    _done = []

    def _ckpt(n):
        if not _done and stop_at <= n:
            _done.append(n)

    def _truncated():
        return bool(_done)

    warmup(int(_os.environ.get("K_WARMUP", "52")))
    for j in (0, 1):
        for cb in (0, 1):
            emit_qk_block(j, 0, cb)
        for cb in (0, 1):
            emit_qk_block(j, 1, cb)
    _ckpt(1)
    if not _truncated():
        for t in range(8):
            emit_v(t)
    _ckpt(2)
    if not _truncated():
        for j in (0, 1):
            for cb in (0, 1):
                emit_fp_block(j, 0, cb)
            for cb in (0, 1):
                emit_fp_block(j, 1, cb)
            emit_dcls(j)
            for tl in (qT[j], kT[j]):
                for cb in (0, 1):
                    emit_rope_block(tl, cb)
    _ckpt(3)

    def next_pair_fillers(jn):
        fs = []
        for cb in (0, 1):
            fs.append(lambda j=jn, c=cb: emit_qk_block(j, 0, c))
        for cb in (0, 1):
            fs.append(lambda j=jn, c=cb: emit_qk_block(j, 1, c))
        for cb in (0, 1):
            fs.append(lambda j=jn, c=cb: emit_fp_block(j, 0, c))
        for cb in (0, 1):
            fs.append(lambda j=jn, c=cb: emit_fp_block(j, 1, c))
        fs.append(lambda j=jn: emit_dcls(j))
        for w in (0, 1):
            for cb in (0, 1):
                fs.append(lambda j=jn, ww=w, c=cb:
                          emit_rope_block(qT[j] if ww == 0 else kT[j], c))
        return fs

    for j in range(6):
        if _truncated():
            break
        fillers = []
        if j + 2 < 6:
            fillers += next_pair_fillers(j + 2)
        if j == 4:
            for jj in range(6):
                fillers.append(lambda jj=jj: emit_pall_gather(jj))
            fillers.append(emit_pall_upcast)
            for t in range(8):
                fillers.append(lambda t=t: emit_ptrans(t))
            fillers.append(emit_pcls_trans)
        if j == 5:
            fillers.append(lambda: emit_ctail(0))
            fillers.append(lambda: emit_ctail(1))
        if j >= 1:
            for cb in (0, 1):
                fillers.append(lambda jj=j - 1, c=cb: emit_norm_block(jj, c))
        emit_B(j, fillers)
        _ckpt(4 + j)

    # ---- tail: cls outputs, last norms, projection ----
    _ckpt(10)
    if _truncated():
        dt_ = main.tile([128, NP], F32, name="dm")
        nc.vector.memset(dt_[:], 0.0)
        for od in range(6):
            nc.sync.dma_start(outT_d[od * 128:(od + 1) * 128, :], dt_[:])
        nc.sync.dma_start(outc_d[:], dt_[0:1, 0:DIM])
        ctx.close()
        return
    for cb in (0, 1):
        emit_norm_block(5, cb)
    # clsrow [12,64] -> transpose -> oT[:, 1024] columns
    psc = fill()
    nc.tensor.transpose(psc[0:64, 0:12], clsrow[0:12, 0:64], identt[0:12, 0:12])
    for h in range(12):
        hj, hp = h // 2, 64 * (h % 2)
        nc.vector.tensor_copy(oT[hj][hp:hp + 64, NP:N], psc[0:64, h:h + 1])
    # normalize cls column: multiply by broadcast 1/dcls per pair
    nc.vector.reciprocal(dclsr2[0:2, 0:6], dclsr[0:2, 0:6])
    dclsb = main.tile([2, 6], BF16, name="dclsb")
    nc.vector.tensor_copy(dclsb[0:2, 0:6], dclsr2[0:2, 0:6])
    for j in range(6):
        ps = fill()
        nc.tensor.matmul(ps[:, 0:1], selt[:, :], dclsb[0:2, j:j + 1],
                         start=True, stop=True)
        nc.vector.tensor_mul(oT[j][:, NP:N], oT[j][:, NP:N], ps[:, 0:1])
    # projection, transposed: outT[od, tok] = sum_kc wpT[kc][:,od]^T @ oT[kc]
    for od in range(6):
        pe = psE.tile([128, 1024], F32, name="pe", tag="eps", bufs=2)
        for cb in (0, 1):
            for kc in range(6):
                nc.tensor.matmul(pe[:, cb * 512:cb * 512 + 512],
                                 wpT[kc][:, od * 128:od * 128 + 128],
                                 oT[kc][:, cb * 512:cb * 512 + 512],
                                 start=(kc == 0), stop=(kc == 5))
        osb = osp.tile([128, NP], F32, name="osb", tag="osb", bufs=2)
        nc.scalar.activation(osb[:, :], pe[:, :], IDN, bias=biat[:, od:od + 1])
        nc.sync.dma_start(outT_d[od * 128:(od + 1) * 128, :], osb[:, :])
    # cls token output row: o_cls @ Wp^T + b
    ocs = osp.tile([1, DIM], F32, name="ocs", tag="ocs", bufs=1)
    for (c0, cw) in ((0, 512), (512, 256)):
        ps = fill()
        for kc in range(6):
            nc.tensor.matmul(ps[0:1, 0:cw], oT[kc][:, NP:N],
                             wpT[kc][:, c0:c0 + cw],
                             start=(kc == 0), stop=(kc == 5))
        nc.vector.tensor_add(ocs[0:1, c0:c0 + cw], ps[0:1, 0:cw],
                             bprt[0:1, c0:c0 + cw])
    nc.sync.dma_start(outc_d[:], ocs[:])

    ctx.close()


def _noop():
    pass


def _build():
    nc = bacc.Bacc(trn_type="TRN2", target_bir_lowering=False)
    with tile.TileContext(nc) as tc:
        _build_body(tc)
    nc.finalize()
    return nc


def _host_tables(xpos_b):
    py = xpos_b[1:, 0].astype(np.float64)
    px = xpos_b[1:, 1].astype(np.float64)
    inv = 1.0 / (100.0 ** (np.arange(0, 32, 2, dtype=np.float64) / 32.0))
    angy = inv[:, None] * py[None, :]
    angx = inv[:, None] * px[None, :]
    c64 = np.concatenate([np.cos(angy), np.cos(angy), np.cos(angx), np.cos(angx)], 0)
    s64 = np.concatenate([np.sin(angy), np.sin(angy), np.sin(angx), np.sin(angx)], 0)
    c128 = np.concatenate([c64, c64], 0)
    s128 = np.concatenate([s64, s64], 0)
    bf = ml_dtypes.bfloat16
    return (np.ascontiguousarray(c128.astype(bf)),
            np.ascontiguousarray(s128.astype(bf)))


def _pmat2():
    P = np.zeros((64, 64), np.float32)
    for i in range(16):
        P[i, i + 16] = -1.0
        P[i + 16, i] = 1.0
        P[i + 32, i + 48] = -1.0
        P[i + 48, i + 32] = 1.0
    P2 = np.zeros((128, 128), np.float32)
    P2[:64, :64] = P
    P2[64:, 64:] = P
    return np.ascontiguousarray(P2.T.astype(ml_dtypes.bfloat16))


def kernel(**inputs):
    bf = ml_dtypes.bfloat16
    x = np.asarray(inputs["x"], np.float32)            # [8,1025,768]
    xpos = np.asarray(inputs["xpos"])                  # [8,1025,2]
    w_qkv = np.asarray(inputs["w_qkv"], np.float32)
    w_proj = np.asarray(inputs["w_proj"], np.float32)
    b_proj = np.asarray(inputs["b_proj"], np.float32).reshape(DIM)
    num_cls = int(np.asarray(inputs["num_cls"]))
    assert num_cls == 1, f"kernel specialized for num_cls=1, got {num_cls}"

    if "nc" not in _CACHE:
        _CACHE["nc"] = _build()
    nc = _CACHE["nc"]

    # shared (batch-independent) host tensors
    wqk = np.ascontiguousarray(w_qkv[0:1536].T.astype(bf))          # [768,1536]
    wv_t = w_qkv[1536:2304].T                                        # [768(in),768]
    wvp = np.zeros((DIM, 792), np.float32)
    for h in range(12):
        wvp[:, h * 66:h * 66 + 64] = wv_t[:, h * 64:(h + 1) * 64]
    wvp = np.ascontiguousarray(wvp.astype(bf))
    wp = np.ascontiguousarray(w_proj.T.astype(bf))                   # [768,768]
    pm2 = _pmat2()
    ident = np.ascontiguousarray(np.eye(128, dtype=np.float32))
    identb_h = np.ascontiguousarray(np.eye(128, dtype=np.float32).astype(bf))
    sel2 = np.zeros((2, 128), np.float32)
    sel2[0, 0:64] = 1.0
    sel2[1, 64:128] = 1.0
    sel2 = np.ascontiguousarray(sel2.astype(bf))
    selab_h = np.zeros((2, 97, 128), np.float32)
    selab_h[0, 0, 0:64] = 1.0    # qh0: head A rows
    selab_h[0, 32, 64:128] = 1.0
    selab_h[1, 64, 0:64] = 1.0   # qh1
    selab_h[1, 96, 64:128] = 1.0
    selab_h = np.ascontiguousarray(selab_h.astype(bf))
    ones12 = np.ones((128, 12), bf)
    biac = np.ascontiguousarray(b_proj.reshape(6, 128).T.astype(np.float32))
    bprow = np.ascontiguousarray(b_proj.reshape(1, DIM).astype(np.float32))

    in_maps = []
    for b in range(NC):
        c128, s128 = _host_tables(xpos[b])
        xt = np.ascontiguousarray(x[b, 1:, :].T.astype(bf))          # [768,1024]
        # host cls projections (f32)
        qkv_cls = w_qkv @ x[b, 0]                                    # [2304]
        q_cls, k_cls, v_cls = qkv_cls[0:768], qkv_cls[768:1536], qkv_cls[1536:2304]
        kcb = np.zeros((128, 12), np.float32)
        qcb = np.zeros((128, 12), np.float32)
        for h in range(12):
            hp = (h % 2) * 64
            kcb[hp:hp + 64, h] = k_cls[h * 64:(h + 1) * 64]
            qcb[hp:hp + 64, h] = q_cls[h * 64:(h + 1) * 64]
        vcr = np.zeros((1, 792), np.float32)
        vcm = np.zeros((2, 792), np.float32)
        for h in range(12):
            vcr[0, h * 66:h * 66 + 64] = v_cls[h * 64:(h + 1) * 64]
            vcr[0, h * 66 + 64] = 1.0
            vcm[h % 2, h * 66:h * 66 + 64] = v_cls[h * 64:(h + 1) * 64]
            vcm[h % 2, h * 66 + 64] = 1.0
        pcc = np.exp(SCALE * (q_cls.reshape(12, 64) * k_cls.reshape(12, 64)).sum(1))
        in_maps.append({
            "xt": xt, "wqk": wqk, "wv": wvp, "wp": wp,
            "ct": c128, "st": s128, "pm": pm2, "ident": ident,
            "kcb": np.ascontiguousarray(kcb.astype(bf)),
            "qcb": np.ascontiguousarray(qcb.astype(bf)),
            "vcm": np.ascontiguousarray(vcm.astype(bf)),
            "vcr": np.ascontiguousarray(vcr.astype(bf)),
            "identb": identb_h, "sel2": sel2, "selab": selab_h, "ones12": ones12,
            "pcc": np.ascontiguousarray(pcc.reshape(12, 1).astype(bf)),
            "biac": biac, "bprow": bprow,
        })
    res = run_bass_kernel_spmd(nc, in_maps, core_ids=list(range(NC)),
                               trace=bool(int(__import__("os").environ.get("BASS_TRACE_KERNEL", "0"))))
    _CACHE["last_result"] = res
    out = np.empty((NC, N, DIM), np.float32)
    for b, r in enumerate(res.results):
        out[b, 1:N, :] = r["outT"].T
        out[b, 0, :] = r["outc"][0]
    return out


# revision 33
# speedup vs baseline: 1.0209x; 1.0209x over previous
"""Trainium2 Bass kernel: ViT attention block with 2D RoPE (croco-style).

Full inputs -> full outputs. Sharding: data-parallel over batch, one batch
element per NeuronCore (B=8 across 8 cores), no collectives.

v3: software-pipelined head pairs.
  - Host-side transposes (x^T, w^T) -> straight DMAs, no xbar transposes.
  - Host-side cls-token qkv projection (0.02% of FLOPs) -> patch-aligned
    tiling everywhere; no 1-column matmul leftovers.
  - Per head pair j: qkv -> fix/pall rows -> rope -> attention, with the
    next pair's matmuls interleaved into pair j's attention loop as PE
    filler so the tensor engine never idles (keeps max p-state).
  - exp on ACT engine only; copies on Pool (gpsimd); rope/normalize on DVE.
  - One reciprocal_approx_fast per pair on [2,1025] (vs 12x [1,1024] full
    Newton reciprocal).
  - Projection computed transposed (out^T = Wp @ o^T) with free-dim-512
    blocks; host re-transposes the [768,1024] result.
"""

import numpy as np
import ml_dtypes

import concourse.bass as bass
import concourse.mybir as mybir
import concourse.tile as tile
from concourse import bacc
from concourse.bass_utils import run_bass_kernel_spmd

F32 = mybir.dt.float32
BF16 = mybir.dt.bfloat16
EXP = mybir.ActivationFunctionType.Exp
IDN = mybir.ActivationFunctionType.Identity
CPY = mybir.ActivationFunctionType.Copy

DIM = 768
H = 12
HD = 64
N = 1025
NP = 1024    # patch tokens
NC = 8
SCALE = HD ** -0.5

_CACHE = {}


def _build_body(tc):
    nc = tc.nc
    import contextlib, os as _os
    ctx = contextlib.ExitStack()
    pool_eng = nc.vector if _os.environ.get("K_NOPOOL", "0") == "1" else nc.gpsimd
    safe_recip = _os.environ.get("K_SAFERECIP", "0") == "1"

    def _recip(out_ap, in_ap):
        if safe_recip:
            nc.vector.reciprocal(out_ap, in_ap)
        else:
            nc.vector.reciprocal_approx_fast(out=out_ap, in_=in_ap)

    # ---- DRAM tensors (all host-prepped layouts) ----
    xt_d = nc.dram_tensor("xt", [DIM, NP], BF16, kind="ExternalInput")      # x[1:].T
    wqk_d = nc.dram_tensor("wqk", [DIM, 1536], BF16, kind="ExternalInput")  # w_qkv[0:1536].T
    wv_d = nc.dram_tensor("wv", [DIM, 792], BF16, kind="ExternalInput")     # w_v.T head-strided
    wp_d = nc.dram_tensor("wp", [DIM, DIM], BF16, kind="ExternalInput")     # w_proj.T
    ct_d = nc.dram_tensor("ct", [128, NP], BF16, kind="ExternalInput")
    st_d = nc.dram_tensor("st", [128, NP], BF16, kind="ExternalInput")
    pm_d = nc.dram_tensor("pm", [128, 128], BF16, kind="ExternalInput")
    id_d = nc.dram_tensor("ident", [128, 128], F32, kind="ExternalInput")
    idb_d = nc.dram_tensor("identb", [128, 128], BF16, kind="ExternalInput")
    kcb_d = nc.dram_tensor("kcb", [128, 12], BF16, kind="ExternalInput")    # k_cls packed
    qcb_d = nc.dram_tensor("qcb", [128, 12], BF16, kind="ExternalInput")    # q_cls packed
    vcm_d = nc.dram_tensor("vcm", [2, 792], BF16, kind="ExternalInput")     # v_cls parity
    vcr_d = nc.dram_tensor("vcr", [1, 792], BF16, kind="ExternalInput")     # v_cls row
    sel_d = nc.dram_tensor("sel2", [2, 128], BF16, kind="ExternalInput")
    selab_d = nc.dram_tensor("selab", [2, 97, 128], BF16, kind="ExternalInput")
    on_d = nc.dram_tensor("ones12", [128, 12], BF16, kind="ExternalInput")
    pcc_d = nc.dram_tensor("pcc", [12, 1], BF16, kind="ExternalInput")      # exp(s*qc.kc)
    bia_d = nc.dram_tensor("biac", [128, 6], F32, kind="ExternalInput")     # b_proj chunks
    bpr_d = nc.dram_tensor("bprow", [1, DIM], F32, kind="ExternalInput")    # b_proj row
    outT_d = nc.dram_tensor("outT", [DIM, NP], F32, kind="ExternalOutput")
    outc_d = nc.dram_tensor("outc", [1, DIM], F32, kind="ExternalOutput")

    # ---- persistent SBUF ----
    const = ctx.enter_context(tc.tile_pool(name="const", bufs=1))
    pmt = const.tile([128, 128], BF16, name="pmt")
    ctt = const.tile([128, NP], BF16, name="ctt")
    stt = const.tile([128, NP], BF16, name="stt")
    identt = const.tile([128, 128], F32, name="identt")
    identb = const.tile([128, 128], BF16, name="identb")
    kcbt = const.tile([128, 12], BF16, name="kcbt")
    qcbt = const.tile([128, 12], BF16, name="qcbt")
    vcmt = const.tile([2, 792], BF16, name="vcmt")
    vcrt = const.tile([1, 792], BF16, name="vcrt")
    selt = const.tile([2, 128], BF16, name="selt")
    selab0 = const.tile([97, 128], BF16, name="selab0")
    selab1 = const.tile([97, 128], BF16, name="selab1")
    biat = const.tile([128, 6], F32, name="biat")
    bprt = const.tile([1, DIM], F32, name="bprt")

    main = ctx.enter_context(tc.tile_pool(name="main", bufs=1))
    xT = [main.tile([128, NP], BF16, name=f"xT{j}") for j in range(6)]
    wqkT = [main.tile([128, 1536], BF16, name=f"wqkT{j}") for j in range(6)]
    wvT = [main.tile([128, 792], BF16, name=f"wvT{j}") for j in range(6)]
    wpT = [main.tile([128, DIM], BF16, name=f"wpT{j}") for j in range(6)]
    qT = [main.tile([128, NP], BF16, name=f"qT{j}") for j in range(6)]
    kT = [main.tile([128, NP], BF16, name=f"kT{j}") for j in range(6)]
    vA = [main.tile([128, 792], BF16, name=f"vA{t}") for t in range(8)]
    oT = [main.tile([128, N], BF16, name=f"oT{j}") for j in range(6)]
    # per-pair row data lives on partitions 0:2, column-concatenated by pair
    # (PE matmul operands require base partition in {0,32,64})
    fixE = main.tile([2, 6 * NP], BF16, name="fixE")
    pall = main.tile([2, 6 * N], BF16, name="pall")
    # denominators: head A on partition 0, head B on partition 32 (engine
    # APs need 32-aligned partition bases); pair j at cols j*1024.
    # rows {0,64}: head A qh0/qh1; rows {32,96}: head B qh0/qh1 (32-aligned)
    den2 = main.tile([97, 6 * 512], F32, name="den2")
    den2b = main.tile([97, 6 * 512], BF16, name="den2b")
    dclsr = main.tile([2, 6], F32, name="dclsr")
    dclsr2 = main.tile([2, 6], F32, name="dclsr2")
    recs = main.tile([97, 512], F32, name="recs")
    ctsb = main.tile([12, 792], F32, name="ctsb")
    pallGb = main.tile([12, N], BF16, name="pallGb")
    pallG = main.tile([12, N], F32, name="pallG")
    pT = [main.tile([128, 12], BF16, name=f"pT{t}") for t in range(8)]
    pclsTs = main.tile([1, 12], BF16, name="pclsTs")
    dclsb = main.tile([2, 6], BF16, name="dclsb")
    clsrow = main.tile([12, 64], F32, name="clsrow")

    etp = ctx.enter_context(tc.tile_pool(name="etp", bufs=1))
    rtp = ctx.enter_context(tc.tile_pool(name="rtp", bufs=1))
    osp = ctx.enter_context(tc.tile_pool(name="osp", bufs=1))

    # PSUM pools: fill 2x[128,512] (2 banks) + eps 2x[128,1024] (4 banks)
    # + oacc 1x[66,1024] (2 banks) = 8 banks.
    psF = ctx.enter_context(tc.tile_pool(name="psF", bufs=1, space="PSUM"))
    psE = ctx.enter_context(tc.tile_pool(name="psE", bufs=1, space="PSUM"))
    psO = ctx.enter_context(tc.tile_pool(name="psO", bufs=1, space="PSUM"))

    def fill():
        return psF.tile([128, 512], F32, name="fl", tag="fl", bufs=2)

    # ---- DMAs, ordered so early compute unblocks first ----
    nc.sync.dma_start(pmt[:], pm_d[:])
    nc.sync.dma_start(ctt[:], ct_d[:])
    nc.sync.dma_start(stt[:], st_d[:])
    for j in range(6):
        nc.sync.dma_start(xT[j][:], xt_d[j * 128:(j + 1) * 128, :])
    for j in range(6):  # q/k cols for pairs 0,1 first
        nc.sync.dma_start(wqkT[j][:, 0:256], wqk_d[j * 128:(j + 1) * 128, 0:256])
        nc.sync.dma_start(wqkT[j][:, 768:1024], wqk_d[j * 128:(j + 1) * 128, 768:1024])
    nc.sync.dma_start(identt[:], id_d[:])
    nc.sync.dma_start(identb[:], idb_d[:])
    nc.sync.dma_start(kcbt[:], kcb_d[:])
    nc.sync.dma_start(qcbt[:], qcb_d[:])
    nc.sync.dma_start(vcmt[:], vcm_d[:])
    nc.sync.dma_start(vcrt[:], vcr_d[:])
    nc.sync.dma_start(selt[:], sel_d[:])
    nc.sync.dma_start(selab0[:], selab_d[0])
    nc.sync.dma_start(selab1[:], selab_d[1])
    nc.sync.dma_start(biat[:], bia_d[:])
    nc.sync.dma_start(bprt[:], bpr_d[:])
    for j in range(6):
        nc.sync.dma_start(pall[0:2, j * N + NP:j * N + N], pcc_d[2 * j:2 * j + 2, 0:1])
    for j in range(6):
        nc.sync.dma_start(wvT[j][:], wv_d[j * 128:(j + 1) * 128, :])
    for j in range(6):
        nc.sync.dma_start(wqkT[j][:, 256:768], wqk_d[j * 128:(j + 1) * 128, 256:768])
        nc.sync.dma_start(wqkT[j][:, 1024:1536], wqk_d[j * 128:(j + 1) * 128, 1024:1536])
    for j in range(6):
        nc.sync.dma_start(wpT[j][:], wp_d[j * 128:(j + 1) * 128, :])

    nc.vector.memset(den2b[0:97, :], 0.0)
    nc.vector.memset(den2[0:97, :], 0.0)

    # ---- emission helpers ----
    def warmup(n):
        for _ in range(n):
            ps = fill()
            nc.tensor.matmul(ps[:, 0:512], pmt[:], ctt[:, 0:512],
                             start=True, stop=True)

    def emit_v(t):
        # v rows for patch tokens t*128..t*128+127 -> vA[t] (head-strided
        # cols; cols 64,65 mod 66 are zero from the padded weight).
        for (c0, cw) in ((0, 512), (512, 280)):
            ps = fill()
            for kc in range(6):
                nc.tensor.matmul(ps[:, 0:cw],
                                 xT[kc][:, t * 128:(t + 1) * 128],
                                 wvT[kc][:, c0:c0 + cw],
                                 start=(kc == 0), stop=(kc == 5))
            nc.scalar.copy(vA[t][:, c0:c0 + cw], ps[:, 0:cw])
        nc.sync.dma_start(vA[t][:, 64::66], on_d[:, 0:12])

    def emit_qk_block(j, which, cb):
        # q^T (which=0) or k^T (which=1) tile j, patch column block cb.
        dst = qT[j] if which == 0 else kT[j]
        m0 = j * 128 if which == 0 else 768 + j * 128
        ps = fill()
        for kc in range(6):
            nc.tensor.matmul(ps[:, 0:512],
                             wqkT[kc][:, m0:m0 + 128],
                             xT[kc][:, cb * 512:cb * 512 + 512],
                             start=(kc == 0), stop=(kc == 5))
        nc.vector.tensor_copy(dst[:, cb * 512:cb * 512 + 512], ps[:, 0:512])

    def emit_fp_block(j, which, cb):
        # fix rows (which=0: k_cls . q_patch -> fixE) or pall rows
        # (which=1: q_cls . k_patch -> pall), heads 2j,2j+1, col block cb.
        cstat = kcbt if which == 0 else qcbt
        rhs = qT[j] if which == 0 else kT[j]
        ps = fill()
        nc.tensor.matmul(ps[0:2, 0:512],
                         cstat[:, 2 * j:2 * j + 2],
                         rhs[:, cb * 512:cb * 512 + 512],
                         start=True, stop=True)
        if which == 0:
            nc.scalar.activation(fixE[0:2, j * NP + cb * 512:j * NP + cb * 512 + 512],
                                 ps[0:2, 0:512], EXP, scale=SCALE)
        else:
            nc.scalar.activation(pall[0:2, j * N + cb * 512:j * N + cb * 512 + 512],
                                 ps[0:2, 0:512], EXP, scale=SCALE)

    def emit_dcls(j):
        # row sums of exp'd pall rows (incl. cls-key col) -> dclsr col j
        nc.vector.tensor_reduce(out=dclsr[0:2, j:j + 1],
                                in_=pall[0:2, j * N:j * N + N],
                                op=mybir.AluOpType.add, axis=mybir.AxisListType.X)

    def emit_rope_block(tl, cb):
        ps = fill()
        nc.tensor.matmul(ps[:, 0:512], pmt[:], tl[:, cb * 512:cb * 512 + 512],
                         start=True, stop=True)
        tmp = rtp.tile([128, 512], BF16, name="rt", tag="rt", bufs=2)
        nc.vector.tensor_mul(tmp[:, :], ps[:, 0:512], stt[:, cb * 512:cb * 512 + 512])
        pool_eng.tensor_mul(tl[:, cb * 512:cb * 512 + 512],
                             tl[:, cb * 512:cb * 512 + 512],
                             ctt[:, cb * 512:cb * 512 + 512])
        nc.vector.tensor_add(tl[:, cb * 512:cb * 512 + 512],
                             tl[:, cb * 512:cb * 512 + 512], tmp[:, :])

    def emit_norm_block(j, cb):
        # rb = broadcast of 1/denom rows (heads 2j,2j+1, this qh) -> [128,512]
        db = j * 512
        sel = selab0 if cb == 0 else selab1
        ps = fill()
        nc.tensor.matmul(ps[:, 0:512], sel[0:97, :], den2b[0:97, db:db + 512],
                         start=True, stop=True)
        nc.vector.tensor_mul(oT[j][:, cb * 512:cb * 512 + 512],
                             oT[j][:, cb * 512:cb * 512 + 512], ps[:, 0:512])

    def emit_pall_gather(j):
        nc.sync.dma_start(pallGb[2 * j:2 * j + 2, 0:N], pall[0:2, j * N:j * N + N])

    def emit_pall_upcast():
        pool_eng.tensor_copy(pallG[0:12, 0:N], pallGb[0:12, 0:N])

    def emit_ptrans(t):
        ps = fill()
        nc.tensor.transpose(ps[0:128, 0:12], pallG[0:12, t * 128:(t + 1) * 128],
                            identt[0:12, 0:12])
        nc.vector.tensor_copy(pT[t][:, 0:12], ps[0:128, 0:12])

    def emit_pcls_trans():
        ps = fill()
        nc.tensor.transpose(ps[0:1, 0:12], pallG[0:12, NP:N], identt[0:12, 0:12])
        nc.vector.tensor_copy(pclsTs[0:1, 0:12], ps[0:1, 0:12])

    def emit_ctail(chunk):
        # cls-query attn@v: ctail[12, 792] = sum_t pT[t]^T@vA[t] + pcls^T@vcr
        c0, cw = (0, 512) if chunk == 0 else (512, 280)
        ps = fill()
        for t in range(8):
            nc.tensor.matmul(ps[0:12, 0:cw], pT[t][:, 0:12], vA[t][:, c0:c0 + cw],
                             start=(t == 0), stop=False, skip_group_check=True)
        nc.tensor.matmul(ps[0:12, 0:cw], pclsTs[0:1, 0:12], vcrt[0:1, c0:c0 + cw],
                         start=False, stop=True, skip_group_check=True)  # cls key
        nc.vector.tensor_copy(ctsb[0:12, c0:c0 + cw], ps[0:12, 0:cw])
        if chunk == 1:
            for h in range(12):
                nc.sync.dma_start(clsrow[h:h + 1, 0:64],
                                  ctsb[h:h + 1, h * 66:h * 66 + 64])

    # ---- stage B (attention) for pair j, with interleaved fillers ----
    def emit_B(j, fillers):
        hA, hB = 2 * j, 2 * j + 1
        its = [(qh, t) for qh in (0, 1) for t in range(8)]
        eps_t = {}
        et_t = {}

        def S(i):
            qh, t = its[i]
            ps = psE.tile([128, 1024], F32, name="eps", tag="eps", bufs=2)
            nc.tensor.matmul(ps[:, 0:512],
                             kT[j][0:64, t * 128:(t + 1) * 128],
                             qT[j][0:64, qh * 512:qh * 512 + 512],
                             start=True, stop=True)
            nc.tensor.matmul(ps[:, 512:1024],
                             kT[j][64:128, t * 128:(t + 1) * 128],
                             qT[j][64:128, qh * 512:qh * 512 + 512],
                             start=True, stop=True)
            et = etp.tile([128, 1024], BF16, name="et", tag="et", bufs=4)
            nc.scalar.activation(et[:, :], ps[:, :], EXP, scale=SCALE)
            eps_t[i] = ps
            et_t[i] = et

        import os as _os
        _ilv = _os.environ.get("K_ILV", "1") == "1"
        if not _ilv:
            for f in fillers:
                f()
            fillers = []
        S(0)
        S(1)
        oacc = None
        fi = 0
        nfill = len(fillers)
        for i, (qh, t) in enumerate(its):
            if i + 2 < 16:
                S(i + 2)
            # drain a fair share of fillers
            want = (i + 1) * nfill // 16
            while fi < want:
                fillers[fi]()
                fi += 1
            if t == 0:
                oacc = psO.tile([66, 1024], F32, name="oacc", tag="oacc", bufs=1)
            et = et_t.pop(i)
            nc.tensor.matmul(oacc[:, 0:512], vA[t][:, hA * 66:hA * 66 + 66],
                             et[:, 0:512],
                             start=(t == 0), stop=False, skip_group_check=True)
            nc.tensor.matmul(oacc[:, 512:1024], vA[t][:, hB * 66:hB * 66 + 66],
                             et[:, 512:1024],
                             start=(t == 0), stop=False, skip_group_check=True)
            if t == 7:
                # cls-key contribution (v_cls parity block x fixE rows)
                fb = j * NP + qh * 512
                nc.tensor.matmul(oacc[:, 0:512],
                                 vcmt[0:2, hA * 66:hA * 66 + 66],
                                 fixE[0:2, fb:fb + 512],
                                 start=False, stop=True, skip_group_check=True)
                nc.tensor.matmul(oacc[:, 512:1024],
                                 vcmt[0:2, hB * 66:hB * 66 + 66],
                                 fixE[0:2, fb:fb + 512],
                                 start=False, stop=True, skip_group_check=True)
                # flush: outputs + denominators
                db = j * 512
                ra = 0 if qh == 0 else 64
                rb_ = 32 if qh == 0 else 96
                nc.vector.tensor_copy(oT[j][0:64, qh * 512:qh * 512 + 512],
                                      oacc[0:64, 0:512])
                nc.vector.tensor_copy(oT[j][64:128, qh * 512:qh * 512 + 512],
                                      oacc[0:64, 512:1024])
                nc.vector.tensor_copy(den2[ra:ra + 1, db:db + 512],
                                      oacc[64:65, 0:512])
                nc.vector.tensor_copy(den2[rb_:rb_ + 1, db:db + 512],
                                      oacc[64:65, 512:1024])
        while fi < nfill:
            fillers[fi]()
            fi += 1
        # reciprocal of this pair's denominators
        jb = j * NP
        # one plain reciprocal across all 4 denominator rows, then casts
        jb2 = j * 512
        nc.vector.reciprocal(recs[0:97, 0:512], den2[0:97, jb2:jb2 + 512])
        cast_eng = nc.vector if j == 5 else pool_eng
        for r in (0, 32, 64, 96):
            cast_eng.tensor_copy(den2b[r:r + 1, jb2:jb2 + 512],
                                 recs[r:r + 1, 0:512])

    def emit_cls_finish():
        # clsrow -> transpose -> oT cls columns, then normalize by 1/dcls
        psc = fill()
        nc.tensor.transpose(psc[0:64, 0:12], clsrow[0:12, 0:64],
                            identt[0:12, 0:12])
        for h in range(12):
            hj, hp = h // 2, 64 * (h % 2)
            nc.vector.tensor_copy(oT[hj][hp:hp + 64, NP:N], psc[0:64, h:h + 1])
        nc.vector.reciprocal(dclsr2[0:2, 0:6], dclsr[0:2, 0:6])
        nc.vector.tensor_copy(dclsb[0:2, 0:6], dclsr2[0:2, 0:6])
        for j in range(6):
            ps = fill()
            nc.tensor.matmul(ps[:, 0:1], selt[:, :], dclsb[0:2, j:j + 1],
                             start=True, stop=True)
            nc.vector.tensor_mul(oT[j][:, NP:N], oT[j][:, NP:N], ps[:, 0:1])

    # ---- full program ----
    stop_at = int(_os.environ.get("K_STOP", "99"))
    _done = []

    def _ckpt(n):
        if not _done and stop_at <= n:
            _done.append(n)

    def _truncated():
        return bool(_done)

    warmup(int(_os.environ.get("K_WARMUP", "36")))
    for j in (0, 1):
        for cb in (0, 1):
            emit_qk_block(j, 0, cb)
        for cb in (0, 1):
            emit_qk_block(j, 1, cb)
    _ckpt(1)
    if not _truncated():
        for t in range(8):
            emit_v(t)
    _ckpt(2)
    if not _truncated():
        for j in (0, 1):
            for cb in (0, 1):
                emit_fp_block(j, 0, cb)
            for cb in (0, 1):
                emit_fp_block(j, 1, cb)
            emit_dcls(j)
            for tl in (qT[j], kT[j]):
                for cb in (0, 1):
                    emit_rope_block(tl, cb)
    _ckpt(3)

    def next_pair_fillers(jn):
        fs = []
        for cb in (0, 1):
            fs.append(lambda j=jn, c=cb: emit_qk_block(j, 0, c))
        for cb in (0, 1):
            fs.append(lambda j=jn, c=cb: emit_qk_block(j, 1, c))
        for cb in (0, 1):
            fs.append(lambda j=jn, c=cb: emit_fp_block(j, 0, c))
        for cb in (0, 1):
            fs.append(lambda j=jn, c=cb: emit_fp_block(j, 1, c))
        fs.append(lambda j=jn: emit_dcls(j))
        for w in (0, 1):
            for cb in (0, 1):
                fs.append(lambda j=jn, ww=w, c=cb:
                          emit_rope_block(qT[j] if ww == 0 else kT[j], c))
        return fs

    for j in range(6):
        if _truncated():
            break
        fillers = []
        if j + 2 < 6:
            fillers += next_pair_fillers(j + 2)
        if j == 4:
            for jj in range(6):
                fillers.append(lambda jj=jj: emit_pall_gather(jj))
            fillers.append(emit_pall_upcast)
            for t in range(8):
                fillers.append(lambda t=t: emit_ptrans(t))
            fillers.append(emit_pcls_trans)
        if j == 5:
            fillers.append(lambda: emit_ctail(0))
            fillers.append(lambda: emit_ctail(1))
        norm_sched = {1: [0], 2: [1], 4: [2, 3], 5: [4]}
        for jj in norm_sched.get(j, []):
            for cb in (0, 1):
                fillers.append(lambda jj=jj, c=cb: emit_norm_block(jj, c))
        if j == 5:
            fillers.append(emit_cls_finish)
        emit_B(j, fillers)
        _ckpt(4 + j)

    # ---- tail: cls outputs, last norms, projection ----
    _ckpt(10)
    if _truncated():
        dt_ = main.tile([128, NP], F32, name="dm")
        nc.vector.memset(dt_[:], 0.0)
        for od in range(6):
            nc.sync.dma_start(outT_d[od * 128:(od + 1) * 128, :], dt_[:])
        nc.sync.dma_start(outc_d[:], dt_[0:1, 0:DIM])
        ctx.close()
        return
    for cb in (0, 1):
        emit_norm_block(5, cb)
    # projection, transposed, split by column half so cb=0 starts right
    # after norm5(cb0)'s multiply
    for cb in (0, 1):
        for od in range(6):
            pe = psE.tile([128, 1024], F32, name="pe", tag="eps", bufs=2)
            for kc in range(6):
                nc.tensor.matmul(pe[:, 0:512],
                                 wpT[kc][:, od * 128:od * 128 + 128],
                                 oT[kc][:, cb * 512:cb * 512 + 512],
                                 start=(kc == 0), stop=(kc == 5))
            osb = osp.tile([128, 512], F32, name="osb", tag="osb", bufs=3)
            nc.scalar.activation(osb[:, 0:512], pe[:, 0:512], IDN,
                                 bias=biat[:, od:od + 1])
            nc.sync.dma_start(
                outT_d[od * 128:(od + 1) * 128, cb * 512:cb * 512 + 512],
                osb[:, 0:512])
    # cls token output row: o_cls @ Wp^T + b
    ocs = osp.tile([1, DIM], F32, name="ocs", tag="ocs", bufs=1)
    for (c0, cw) in ((0, 512), (512, 256)):
        ps = fill()
        for kc in range(6):
            nc.tensor.matmul(ps[0:1, 0:cw], oT[kc][:, NP:N],
                             wpT[kc][:, c0:c0 + cw],
                             start=(kc == 0), stop=(kc == 5))
        nc.vector.tensor_add(ocs[0:1, c0:c0 + cw], ps[0:1, 0:cw],
                             bprt[0:1, c0:c0 + cw])
    nc.sync.dma_start(outc_d[:], ocs[:])

    ctx.close()


def _noop():
    pass


def _build():
    nc = bacc.Bacc(trn_type="TRN2", target_bir_lowering=False)
    with tile.TileContext(nc) as tc:
        _build_body(tc)
    nc.finalize()
    return nc


def _host_tables(xpos_b):
    py = xpos_b[1:, 0].astype(np.float64)
    px = xpos_b[1:, 1].astype(np.float64)
    inv = 1.0 / (100.0 ** (np.arange(0, 32, 2, dtype=np.float64) / 32.0))
    angy = inv[:, None] * py[None, :]
    angx = inv[:, None] * px[None, :]
    c64 = np.concatenate([np.cos(angy), np.cos(angy), np.cos(angx), np.cos(angx)], 0)
    s64 = np.concatenate([np.sin(angy), np.sin(angy), np.sin(angx), np.sin(angx)], 0)
    c128 = np.concatenate([c64, c64], 0)
    s128 = np.concatenate([s64, s64], 0)
    bf = ml_dtypes.bfloat16
    return (np.ascontiguousarray(c128.astype(bf)),
            np.ascontiguousarray(s128.astype(bf)))


def _pmat2():
    P = np.zeros((64, 64), np.float32)
    for i in range(16):
        P[i, i + 16] = -1.0
        P[i + 16, i] = 1.0
        P[i + 32, i + 48] = -1.0
        P[i + 48, i + 32] = 1.0
    P2 = np.zeros((128, 128), np.float32)
    P2[:64, :64] = P
    P2[64:, 64:] = P
    return np.ascontiguousarray(P2.T.astype(ml_dtypes.bfloat16))


def kernel(**inputs):
    bf = ml_dtypes.bfloat16
    x = np.asarray(inputs["x"], np.float32)            # [8,1025,768]
    xpos = np.asarray(inputs["xpos"])                  # [8,1025,2]
    w_qkv = np.asarray(inputs["w_qkv"], np.float32)
    w_proj = np.asarray(inputs["w_proj"], np.float32)
    b_proj = np.asarray(inputs["b_proj"], np.float32).reshape(DIM)
    num_cls = int(np.asarray(inputs["num_cls"]))
    assert num_cls == 1, f"kernel specialized for num_cls=1, got {num_cls}"

    if "nc" not in _CACHE:
        _CACHE["nc"] = _build()
    nc = _CACHE["nc"]

    # shared (batch-independent) host tensors
    wqk = np.ascontiguousarray(w_qkv[0:1536].T.astype(bf))          # [768,1536]
    wv_t = w_qkv[1536:2304].T                                        # [768(in),768]
    wvp = np.zeros((DIM, 792), np.float32)
    for h in range(12):
        wvp[:, h * 66:h * 66 + 64] = wv_t[:, h * 64:(h + 1) * 64]
    wvp = np.ascontiguousarray(wvp.astype(bf))
    wp = np.ascontiguousarray(w_proj.T.astype(bf))                   # [768,768]
    pm2 = _pmat2()
    ident = np.ascontiguousarray(np.eye(128, dtype=np.float32))
    identb_h = np.ascontiguousarray(np.eye(128, dtype=np.float32).astype(bf))
    sel2 = np.zeros((2, 128), np.float32)
    sel2[0, 0:64] = 1.0
    sel2[1, 64:128] = 1.0
    sel2 = np.ascontiguousarray(sel2.astype(bf))
    selab_h = np.zeros((2, 97, 128), np.float32)
    selab_h[0, 0, 0:64] = 1.0    # qh0: head A rows
    selab_h[0, 32, 64:128] = 1.0
    selab_h[1, 64, 0:64] = 1.0   # qh1
    selab_h[1, 96, 64:128] = 1.0
    selab_h = np.ascontiguousarray(selab_h.astype(bf))
    ones12 = np.ones((128, 12), bf)
    biac = np.ascontiguousarray(b_proj.reshape(6, 128).T.astype(np.float32))
    bprow = np.ascontiguousarray(b_proj.reshape(1, DIM).astype(np.float32))

    in_maps = []
    for b in range(NC):
        c128, s128 = _host_tables(xpos[b])
        xt = np.ascontiguousarray(x[b, 1:, :].T.astype(bf))          # [768,1024]
        # host cls projections (f32)
        qkv_cls = w_qkv @ x[b, 0]                                    # [2304]
        q_cls, k_cls, v_cls = qkv_cls[0:768], qkv_cls[768:1536], qkv_cls[1536:2304]
        kcb = np.zeros((128, 12), np.float32)
        qcb = np.zeros((128, 12), np.float32)
        for h in range(12):
            hp = (h % 2) * 64
            kcb[hp:hp + 64, h] = k_cls[h * 64:(h + 1) * 64]
            qcb[hp:hp + 64, h] = q_cls[h * 64:(h + 1) * 64]
        vcr = np.zeros((1, 792), np.float32)
        vcm = np.zeros((2, 792), np.float32)
        for h in range(12):
            vcr[0, h * 66:h * 66 + 64] = v_cls[h * 64:(h + 1) * 64]
            vcr[0, h * 66 + 64] = 1.0
            vcm[h % 2, h * 66:h * 66 + 64] = v_cls[h * 64:(h + 1) * 64]
            vcm[h % 2, h * 66 + 64] = 1.0
        pcc = np.exp(SCALE * (q_cls.reshape(12, 64) * k_cls.reshape(12, 64)).sum(1))
        in_maps.append({
            "xt": xt, "wqk": wqk, "wv": wvp, "wp": wp,
            "ct": c128, "st": s128, "pm": pm2, "ident": ident,
            "kcb": np.ascontiguousarray(kcb.astype(bf)),
            "qcb": np.ascontiguousarray(qcb.astype(bf)),
            "vcm": np.ascontiguousarray(vcm.astype(bf)),
            "vcr": np.ascontiguousarray(vcr.astype(bf)),
            "identb": identb_h, "sel2": sel2, "selab": selab_h, "ones12": ones12,
            "pcc": np.ascontiguousarray(pcc.reshape(12, 1).astype(bf)),
            "biac": biac, "bprow": bprow,
        })
    res = run_bass_kernel_spmd(nc, in_maps, core_ids=list(range(NC)),
                               trace=bool(int(__import__("os").environ.get("BASS_TRACE_KERNEL", "0"))))
    _CACHE["last_result"] = res
    out = np.empty((NC, N, DIM), np.float32)
    for b, r in enumerate(res.results):
        out[b, 1:N, :] = r["outT"].T
        out[b, 0, :] = r["outc"][0]
    return out


# revision 34
# speedup vs baseline: 1.1125x; 1.0897x over previous
"""Trainium2 Bass kernel: ViT attention block with 2D RoPE (croco-style).

Full inputs -> full outputs. Sharding: data-parallel over batch, one batch
element per NeuronCore (B=8 across 8 cores), no collectives.

v3: software-pipelined head pairs.
  - Host-side transposes (x^T, w^T) -> straight DMAs, no xbar transposes.
  - Host-side cls-token qkv projection (0.02% of FLOPs) -> patch-aligned
    tiling everywhere; no 1-column matmul leftovers.
  - Per head pair j: qkv -> fix/pall rows -> rope -> attention, with the
    next pair's matmuls interleaved into pair j's attention loop as PE
    filler so the tensor engine never idles (keeps max p-state).
  - exp on ACT engine only; copies on Pool (gpsimd); rope/normalize on DVE.
  - One reciprocal_approx_fast per pair on [2,1025] (vs 12x [1,1024] full
    Newton reciprocal).
  - Projection computed transposed (out^T = Wp @ o^T) with free-dim-512
    blocks; host re-transposes the [768,1024] result.
"""

import numpy as np
import ml_dtypes

import concourse.bass as bass
import concourse.mybir as mybir
import concourse.tile as tile
from concourse import bacc
from concourse.bass_utils import run_bass_kernel_spmd

F32 = mybir.dt.float32
BF16 = mybir.dt.bfloat16
EXP = mybir.ActivationFunctionType.Exp
IDN = mybir.ActivationFunctionType.Identity
CPY = mybir.ActivationFunctionType.Copy

DIM = 768
H = 12
HD = 64
N = 1025
NP = 1024    # patch tokens
NC = 8
SCALE = HD ** -0.5

_CACHE = {}


def _build_body(tc):
    nc = tc.nc
    import contextlib, os as _os
    ctx = contextlib.ExitStack()
    pool_eng = nc.vector if _os.environ.get("K_NOPOOL", "0") == "1" else nc.gpsimd
    safe_recip = _os.environ.get("K_SAFERECIP", "0") == "1"

    def _recip(out_ap, in_ap):
        if safe_recip:
            nc.vector.reciprocal(out_ap, in_ap)
        else:
            nc.vector.reciprocal_approx_fast(out=out_ap, in_=in_ap)

    # ---- DRAM tensors (all host-prepped layouts) ----
    xt_d = nc.dram_tensor("xt", [DIM, NP], BF16, kind="ExternalInput")      # x[1:].T
    wqk_d = nc.dram_tensor("wqk", [DIM, 1536], BF16, kind="ExternalInput")  # w_qkv[0:1536].T
    wv_d = nc.dram_tensor("wv", [DIM, 792], BF16, kind="ExternalInput")     # w_v.T head-strided
    wp_d = nc.dram_tensor("wp", [DIM, DIM], BF16, kind="ExternalInput")     # w_proj.T
    ct_d = nc.dram_tensor("ct", [128, NP], BF16, kind="ExternalInput")
    st_d = nc.dram_tensor("st", [128, NP], BF16, kind="ExternalInput")
    pm_d = nc.dram_tensor("pm", [128, 128], BF16, kind="ExternalInput")
    id_d = nc.dram_tensor("ident", [128, 128], F32, kind="ExternalInput")
    idb_d = nc.dram_tensor("identb", [128, 128], BF16, kind="ExternalInput")
    kcb_d = nc.dram_tensor("kcb", [128, 12], BF16, kind="ExternalInput")    # k_cls packed
    qcb_d = nc.dram_tensor("qcb", [128, 12], BF16, kind="ExternalInput")    # q_cls packed
    vcm_d = nc.dram_tensor("vcm", [2, 792], BF16, kind="ExternalInput")     # v_cls parity
    vcr_d = nc.dram_tensor("vcr", [1, 792], BF16, kind="ExternalInput")     # v_cls row
    sel_d = nc.dram_tensor("sel2", [2, 128], BF16, kind="ExternalInput")
    selab_d = nc.dram_tensor("selab", [2, 97, 128], BF16, kind="ExternalInput")
    on_d = nc.dram_tensor("ones12", [128, 12], BF16, kind="ExternalInput")
    pcc_d = nc.dram_tensor("pcc", [12, 1], BF16, kind="ExternalInput")      # exp(s*qc.kc)
    bia_d = nc.dram_tensor("biac", [128, 6], F32, kind="ExternalInput")     # b_proj chunks
    bpr_d = nc.dram_tensor("bprow", [1, DIM], F32, kind="ExternalInput")    # b_proj row
    outT_d = nc.dram_tensor("outT", [DIM, NP], F32, kind="ExternalOutput")
    outc_d = nc.dram_tensor("outc", [1, DIM], F32, kind="ExternalOutput")

    # ---- persistent SBUF ----
    const = ctx.enter_context(tc.tile_pool(name="const", bufs=1))
    pmt = const.tile([128, 128], BF16, name="pmt")
    ctt = const.tile([128, NP], BF16, name="ctt")
    stt = const.tile([128, NP], BF16, name="stt")
    identt = const.tile([128, 128], F32, name="identt")
    identb = const.tile([128, 128], BF16, name="identb")
    kcbt = const.tile([128, 12], BF16, name="kcbt")
    qcbt = const.tile([128, 12], BF16, name="qcbt")
    vcmt = const.tile([2, 792], BF16, name="vcmt")
    vcrt = const.tile([1, 792], BF16, name="vcrt")
    selt = const.tile([2, 128], BF16, name="selt")
    selab0 = const.tile([97, 128], BF16, name="selab0")
    selab1 = const.tile([97, 128], BF16, name="selab1")
    biat = const.tile([128, 6], F32, name="biat")
    bprt = const.tile([1, DIM], F32, name="bprt")

    main = ctx.enter_context(tc.tile_pool(name="main", bufs=1))
    xT = [main.tile([128, NP], BF16, name=f"xT{j}") for j in range(6)]
    wqkT = [main.tile([128, 1536], BF16, name=f"wqkT{j}") for j in range(6)]
    wvT = [main.tile([128, 792], BF16, name=f"wvT{j}") for j in range(6)]
    wpT = [main.tile([128, DIM], BF16, name=f"wpT{j}") for j in range(6)]
    qT = [main.tile([128, NP], BF16, name=f"qT{j}") for j in range(6)]
    kT = [main.tile([128, NP], BF16, name=f"kT{j}") for j in range(6)]
    vA = [main.tile([128, 792], BF16, name=f"vA{t}") for t in range(8)]
    oT = [main.tile([128, N], BF16, name=f"oT{j}") for j in range(6)]
    # per-pair row data lives on partitions 0:2, column-concatenated by pair
    # (PE matmul operands require base partition in {0,32,64})
    fixE = main.tile([2, 6 * NP], BF16, name="fixE")
    pall = main.tile([2, 6 * N], BF16, name="pall")
    # denominators: head A on partition 0, head B on partition 32 (engine
    # APs need 32-aligned partition bases); pair j at cols j*1024.
    # rows {0,64}: head A qh0/qh1; rows {32,96}: head B qh0/qh1 (32-aligned)
    den2 = main.tile([97, 6 * 512], F32, name="den2")
    den2b = main.tile([97, 6 * 512], BF16, name="den2b")
    dclsr = main.tile([2, 6], F32, name="dclsr")
    dclsr2 = main.tile([2, 6], F32, name="dclsr2")
    recs = main.tile([97, 512], F32, name="recs")
    ctsb = main.tile([12, 792], F32, name="ctsb")
    pallGb = main.tile([12, N], BF16, name="pallGb")
    pallG = main.tile([12, N], F32, name="pallG")
    pT = [main.tile([128, 12], BF16, name=f"pT{t}") for t in range(8)]
    pclsTs = main.tile([1, 12], BF16, name="pclsTs")
    clsrow = main.tile([12, 64], F32, name="clsrow")

    etp = ctx.enter_context(tc.tile_pool(name="etp", bufs=1))
    rtp = ctx.enter_context(tc.tile_pool(name="rtp", bufs=1))
    osp = ctx.enter_context(tc.tile_pool(name="osp", bufs=1))

    # PSUM pools: fill 2x[128,512] (2 banks) + eps 2x[128,1024] (4 banks)
    # + oacc 1x[66,1024] (2 banks) = 8 banks.
    psF = ctx.enter_context(tc.tile_pool(name="psF", bufs=1, space="PSUM"))
    psE = ctx.enter_context(tc.tile_pool(name="psE", bufs=1, space="PSUM"))
    psO = ctx.enter_context(tc.tile_pool(name="psO", bufs=1, space="PSUM"))

    def fill():
        return psF.tile([128, 512], F32, name="fl", tag="fl", bufs=2)

    # ---- DMAs, ordered so early compute unblocks first ----
    nc.sync.dma_start(pmt[:], pm_d[:])
    nc.sync.dma_start(ctt[:], ct_d[:])
    nc.sync.dma_start(stt[:], st_d[:])
    for j in range(6):
        nc.sync.dma_start(xT[j][:], xt_d[j * 128:(j + 1) * 128, :])
    for j in range(6):  # q/k cols for pairs 0,1 first
        nc.sync.dma_start(wqkT[j][:, 0:256], wqk_d[j * 128:(j + 1) * 128, 0:256])
        nc.sync.dma_start(wqkT[j][:, 768:1024], wqk_d[j * 128:(j + 1) * 128, 768:1024])
    nc.sync.dma_start(identt[:], id_d[:])
    nc.sync.dma_start(identb[:], idb_d[:])
    nc.sync.dma_start(kcbt[:], kcb_d[:])
    nc.sync.dma_start(qcbt[:], qcb_d[:])
    nc.sync.dma_start(vcmt[:], vcm_d[:])
    nc.sync.dma_start(vcrt[:], vcr_d[:])
    nc.sync.dma_start(selt[:], sel_d[:])
    nc.sync.dma_start(selab0[:], selab_d[0])
    nc.sync.dma_start(selab1[:], selab_d[1])
    nc.sync.dma_start(biat[:], bia_d[:])
    nc.sync.dma_start(bprt[:], bpr_d[:])
    for j in range(6):
        nc.sync.dma_start(pall[0:2, j * N + NP:j * N + N], pcc_d[2 * j:2 * j + 2, 0:1])
    for j in range(6):
        nc.sync.dma_start(wvT[j][:], wv_d[j * 128:(j + 1) * 128, :])
    for j in range(6):
        nc.sync.dma_start(wqkT[j][:, 256:768], wqk_d[j * 128:(j + 1) * 128, 256:768])
        nc.sync.dma_start(wqkT[j][:, 1024:1536], wqk_d[j * 128:(j + 1) * 128, 1024:1536])
    for j in range(6):
        nc.sync.dma_start(wpT[j][:], wp_d[j * 128:(j + 1) * 128, :])

    nc.vector.memset(den2b[0:97, :], 0.0)
    nc.vector.memset(den2[0:97, :], 0.0)

    # ---- emission helpers ----
    def warmup(n):
        for _ in range(n):
            ps = fill()
            nc.tensor.matmul(ps[:, 0:512], pmt[:], ctt[:, 0:512],
                             start=True, stop=True)

    def emit_v(t):
        # v rows for patch tokens t*128..t*128+127 -> vA[t] (head-strided
        # cols; cols 64,65 mod 66 are zero from the padded weight).
        for (c0, cw) in ((0, 512), (512, 280)):
            ps = fill()
            for kc in range(6):
                nc.tensor.matmul(ps[:, 0:cw],
                                 xT[kc][:, t * 128:(t + 1) * 128],
                                 wvT[kc][:, c0:c0 + cw],
                                 start=(kc == 0), stop=(kc == 5))
            nc.scalar.copy(vA[t][:, c0:c0 + cw], ps[:, 0:cw])
        nc.sync.dma_start(vA[t][:, 64::66], on_d[:, 0:12])

    def emit_qk_block(j, which, cb):
        # q^T (which=0) or k^T (which=1) tile j, patch column block cb.
        dst = qT[j] if which == 0 else kT[j]
        m0 = j * 128 if which == 0 else 768 + j * 128
        ps = fill()
        for kc in range(6):
            nc.tensor.matmul(ps[:, 0:512],
                             wqkT[kc][:, m0:m0 + 128],
                             xT[kc][:, cb * 512:cb * 512 + 512],
                             start=(kc == 0), stop=(kc == 5))
        nc.vector.tensor_copy(dst[:, cb * 512:cb * 512 + 512], ps[:, 0:512])

    def emit_fp_block(j, which, cb):
        # fix rows (which=0: k_cls . q_patch -> fixE) or pall rows
        # (which=1: q_cls . k_patch -> pall), heads 2j,2j+1, col block cb.
        cstat = kcbt if which == 0 else qcbt
        rhs = qT[j] if which == 0 else kT[j]
        ps = fill()
        nc.tensor.matmul(ps[0:2, 0:512],
                         cstat[:, 2 * j:2 * j + 2],
                         rhs[:, cb * 512:cb * 512 + 512],
                         start=True, stop=True)
        if which == 0:
            nc.scalar.activation(fixE[0:2, j * NP + cb * 512:j * NP + cb * 512 + 512],
                                 ps[0:2, 0:512], EXP, scale=SCALE)
        else:
            nc.scalar.activation(pall[0:2, j * N + cb * 512:j * N + cb * 512 + 512],
                                 ps[0:2, 0:512], EXP, scale=SCALE)

    def emit_dcls(j):
        # row sums of exp'd pall rows (incl. cls-key col) -> dclsr col j
        nc.vector.tensor_reduce(out=dclsr[0:2, j:j + 1],
                                in_=pall[0:2, j * N:j * N + N],
                                op=mybir.AluOpType.add, axis=mybir.AxisListType.X)

    def emit_rope_block(tl, cb):
        ps = fill()
        nc.tensor.matmul(ps[:, 0:512], pmt[:], tl[:, cb * 512:cb * 512 + 512],
                         start=True, stop=True)
        tmp = rtp.tile([128, 512], BF16, name="rt", tag="rt", bufs=2)
        nc.vector.tensor_mul(tmp[:, :], ps[:, 0:512], stt[:, cb * 512:cb * 512 + 512])
        pool_eng.tensor_mul(tl[:, cb * 512:cb * 512 + 512],
                             tl[:, cb * 512:cb * 512 + 512],
                             ctt[:, cb * 512:cb * 512 + 512])
        nc.vector.tensor_add(tl[:, cb * 512:cb * 512 + 512],
                             tl[:, cb * 512:cb * 512 + 512], tmp[:, :])

    def emit_norm_block(j, cb):
        # rb = broadcast of 1/denom rows (heads 2j,2j+1, this qh) -> [128,512]
        db = j * 512
        sel = selab0 if cb == 0 else selab1
        ps = fill()
        nc.tensor.matmul(ps[:, 0:512], sel[0:97, :], den2b[0:97, db:db + 512],
                         start=True, stop=True)
        nc.vector.tensor_mul(oT[j][:, cb * 512:cb * 512 + 512],
                             oT[j][:, cb * 512:cb * 512 + 512], ps[:, 0:512])

    def emit_pall_gather(j):
        nc.sync.dma_start(pallGb[2 * j:2 * j + 2, 0:N], pall[0:2, j * N:j * N + N])

    def emit_pall_upcast():
        pool_eng.tensor_copy(pallG[0:12, 0:N], pallGb[0:12, 0:N])

    def emit_ptrans(t):
        ps = fill()
        nc.tensor.transpose(ps[0:128, 0:12], pallG[0:12, t * 128:(t + 1) * 128],
                            identt[0:12, 0:12])
        nc.vector.tensor_copy(pT[t][:, 0:12], ps[0:128, 0:12])

    def emit_pcls_trans():
        ps = fill()
        nc.tensor.transpose(ps[0:1, 0:12], pallG[0:12, NP:N], identt[0:12, 0:12])
        nc.vector.tensor_copy(pclsTs[0:1, 0:12], ps[0:1, 0:12])

    def emit_ctail(chunk):
        # cls-query attn@v: ctail[12, 792] = sum_t pT[t]^T@vA[t] + pcls^T@vcr
        c0, cw = (0, 512) if chunk == 0 else (512, 280)
        ps = fill()
        for t in range(8):
            nc.tensor.matmul(ps[0:12, 0:cw], pT[t][:, 0:12], vA[t][:, c0:c0 + cw],
                             start=(t == 0), stop=False, skip_group_check=True)
        nc.tensor.matmul(ps[0:12, 0:cw], pclsTs[0:1, 0:12], vcrt[0:1, c0:c0 + cw],
                         start=False, stop=True, skip_group_check=True)  # cls key
        nc.vector.tensor_copy(ctsb[0:12, c0:c0 + cw], ps[0:12, 0:cw])
        if chunk == 1:
            for h in range(12):
                nc.sync.dma_start(clsrow[h:h + 1, 0:64],
                                  ctsb[h:h + 1, h * 66:h * 66 + 64])

    # ---- stage B (attention) for pair j, with interleaved fillers ----
    def emit_B(j, fillers):
        hA, hB = 2 * j, 2 * j + 1
        its = [(qh, t) for qh in (0, 1) for t in range(8)]
        eps_t = {}
        et_t = {}

        def S(i):
            qh, t = its[i]
            ps = psE.tile([128, 1024], F32, name="eps", tag="eps", bufs=2)
            nc.tensor.matmul(ps[:, 0:512],
                             kT[j][0:64, t * 128:(t + 1) * 128],
                             qT[j][0:64, qh * 512:qh * 512 + 512],
                             start=True, stop=True)
            nc.tensor.matmul(ps[:, 512:1024],
                             kT[j][64:128, t * 128:(t + 1) * 128],
                             qT[j][64:128, qh * 512:qh * 512 + 512],
                             start=True, stop=True)
            et = etp.tile([128, 1024], BF16, name="et", tag="et", bufs=4)
            nc.scalar.activation(et[:, :], ps[:, :], EXP, scale=SCALE)
            eps_t[i] = ps
            et_t[i] = et

        import os as _os
        _ilv = _os.environ.get("K_ILV", "1") == "1"
        if not _ilv:
            for f in fillers:
                f()
            fillers = []
        S(0)
        S(1)
        oacc = None
        fi = 0
        nfill = len(fillers)
        for i, (qh, t) in enumerate(its):
            if i + 2 < 16:
                S(i + 2)
            # drain a fair share of fillers
            want = (i + 1) * nfill // 16
            while fi < want:
                fillers[fi]()
                fi += 1
            if t == 0:
                oacc = psO.tile([66, 1024], F32, name="oacc", tag="oacc", bufs=1)
            et = et_t.pop(i)
            nc.tensor.matmul(oacc[:, 0:512], vA[t][:, hA * 66:hA * 66 + 66],
                             et[:, 0:512],
                             start=(t == 0), stop=False, skip_group_check=True)
            nc.tensor.matmul(oacc[:, 512:1024], vA[t][:, hB * 66:hB * 66 + 66],
                             et[:, 512:1024],
                             start=(t == 0), stop=False, skip_group_check=True)
            if t == 7:
                # cls-key contribution (v_cls parity block x fixE rows)
                fb = j * NP + qh * 512
                nc.tensor.matmul(oacc[:, 0:512],
                                 vcmt[0:2, hA * 66:hA * 66 + 66],
                                 fixE[0:2, fb:fb + 512],
                                 start=False, stop=True, skip_group_check=True)
                nc.tensor.matmul(oacc[:, 512:1024],
                                 vcmt[0:2, hB * 66:hB * 66 + 66],
                                 fixE[0:2, fb:fb + 512],
                                 start=False, stop=True, skip_group_check=True)
                # flush: outputs + denominators
                db = j * 512
                ra = 0 if qh == 0 else 64
                rb_ = 32 if qh == 0 else 96
                nc.vector.tensor_copy(oT[j][0:64, qh * 512:qh * 512 + 512],
                                      oacc[0:64, 0:512])
                nc.vector.tensor_copy(oT[j][64:128, qh * 512:qh * 512 + 512],
                                      oacc[0:64, 512:1024])
                nc.vector.tensor_copy(den2[ra:ra + 1, db:db + 512],
                                      oacc[64:65, 0:512])
                nc.vector.tensor_copy(den2[rb_:rb_ + 1, db:db + 512],
                                      oacc[64:65, 512:1024])
        while fi < nfill:
            fillers[fi]()
            fi += 1
        # reciprocal of this pair's denominators
        jb = j * NP
        # one plain reciprocal across all 4 denominator rows, then casts
        jb2 = j * 512
        nc.vector.reciprocal(recs[0:97, 0:512], den2[0:97, jb2:jb2 + 512])
        for r in (0, 32, 64, 96):
            pool_eng.tensor_copy(den2b[r:r + 1, jb2:jb2 + 512],
                                 recs[r:r + 1, 0:512])

    # ---- full program ----
    stop_at = int(_os.environ.get("K_STOP", "99"))
    _done = []

    def _ckpt(n):
        if not _done and stop_at <= n:
            _done.append(n)

    def _truncated():
        return bool(_done)

    warmup(int(_os.environ.get("K_WARMUP", "36")))
    for j in (0, 1):
        for cb in (0, 1):
            emit_qk_block(j, 0, cb)
        for cb in (0, 1):
            emit_qk_block(j, 1, cb)
    _ckpt(1)
    if not _truncated():
        for t in range(8):
            emit_v(t)
    _ckpt(2)
    if not _truncated():
        for j in (0, 1):
            for cb in (0, 1):
                emit_fp_block(j, 0, cb)
            for cb in (0, 1):
                emit_fp_block(j, 1, cb)
            emit_dcls(j)
            for tl in (qT[j], kT[j]):
                for cb in (0, 1):
                    emit_rope_block(tl, cb)
    _ckpt(3)

    def next_pair_fillers(jn):
        fs = []
        for cb in (0, 1):
            fs.append(lambda j=jn, c=cb: emit_qk_block(j, 0, c))
        for cb in (0, 1):
            fs.append(lambda j=jn, c=cb: emit_qk_block(j, 1, c))
        for cb in (0, 1):
            fs.append(lambda j=jn, c=cb: emit_fp_block(j, 0, c))
        for cb in (0, 1):
            fs.append(lambda j=jn, c=cb: emit_fp_block(j, 1, c))
        fs.append(lambda j=jn: emit_dcls(j))
        for w in (0, 1):
            for cb in (0, 1):
                fs.append(lambda j=jn, ww=w, c=cb:
                          emit_rope_block(qT[j] if ww == 0 else kT[j], c))
        return fs

    for j in range(6):
        if _truncated():
            break
        fillers = []
        if j + 2 < 6:
            fillers += next_pair_fillers(j + 2)
        if j == 4:
            for jj in range(6):
                fillers.append(lambda jj=jj: emit_pall_gather(jj))
            fillers.append(emit_pall_upcast)
            for t in range(8):
                fillers.append(lambda t=t: emit_ptrans(t))
            fillers.append(emit_pcls_trans)
        if j == 5:
            fillers.append(lambda: emit_ctail(0))
            fillers.append(lambda: emit_ctail(1))
        if j >= 1:
            for cb in (0, 1):
                fillers.append(lambda jj=j - 1, c=cb: emit_norm_block(jj, c))
        emit_B(j, fillers)
        _ckpt(4 + j)

    # ---- tail: cls outputs, last norms, projection ----
    _ckpt(10)
    if _truncated():
        dt_ = main.tile([128, NP], F32, name="dm")
        nc.vector.memset(dt_[:], 0.0)
        for od in range(6):
            nc.sync.dma_start(outT_d[od * 128:(od + 1) * 128, :], dt_[:])
        nc.sync.dma_start(outc_d[:], dt_[0:1, 0:DIM])
        ctx.close()
        return
    for cb in (0, 1):
        emit_norm_block(5, cb)
    # clsrow [12,64] -> transpose -> oT[:, 1024] columns
    psc = fill()
    nc.tensor.transpose(psc[0:64, 0:12], clsrow[0:12, 0:64], identt[0:12, 0:12])
    for h in range(12):
        hj, hp = h // 2, 64 * (h % 2)
        nc.vector.tensor_copy(oT[hj][hp:hp + 64, NP:N], psc[0:64, h:h + 1])
    # normalize cls column: multiply by broadcast 1/dcls per pair
    nc.vector.reciprocal(dclsr2[0:2, 0:6], dclsr[0:2, 0:6])
    dclsb = main.tile([2, 6], BF16, name="dclsb")
    nc.vector.tensor_copy(dclsb[0:2, 0:6], dclsr2[0:2, 0:6])
    for j in range(6):
        ps = fill()
        nc.tensor.matmul(ps[:, 0:1], selt[:, :], dclsb[0:2, j:j + 1],
                         start=True, stop=True)
        nc.vector.tensor_mul(oT[j][:, NP:N], oT[j][:, NP:N], ps[:, 0:1])
    # projection, transposed: outT[od, tok] = sum_kc wpT[kc][:,od]^T @ oT[kc]
    for od in range(6):
        pe = psE.tile([128, 1024], F32, name="pe", tag="eps", bufs=2)
        for cb in (0, 1):
            for kc in range(6):
                nc.tensor.matmul(pe[:, cb * 512:cb * 512 + 512],
                                 wpT[kc][:, od * 128:od * 128 + 128],
                                 oT[kc][:, cb * 512:cb * 512 + 512],
                                 start=(kc == 0), stop=(kc == 5))
        osb = osp.tile([128, NP], F32, name="osb", tag="osb", bufs=2)
        nc.scalar.activation(osb[:, :], pe[:, :], IDN, bias=biat[:, od:od + 1])
        nc.sync.dma_start(outT_d[od * 128:(od + 1) * 128, :], osb[:, :])
    # cls token output row: o_cls @ Wp^T + b
    ocs = osp.tile([1, DIM], F32, name="ocs", tag="ocs", bufs=1)
    for (c0, cw) in ((0, 512), (512, 256)):
        ps = fill()
        for kc in range(6):
            nc.tensor.matmul(ps[0:1, 0:cw], oT[kc][:, NP:N],
                             wpT[kc][:, c0:c0 + cw],
                             start=(kc == 0), stop=(kc == 5))
        nc.vector.tensor_add(ocs[0:1, c0:c0 + cw], ps[0:1, 0:cw],
                             bprt[0:1, c0:c0 + cw])
    nc.sync.dma_start(outc_d[:], ocs[:])

    ctx.close()


def _noop():
    pass


def _build():
    nc = bacc.Bacc(trn_type="TRN2", target_bir_lowering=False)
    with tile.TileContext(nc) as tc:
        _build_body(tc)
    nc.finalize()
    return nc


def _host_tables(xpos_b):
    py = xpos_b[1:, 0].astype(np.float64)
    px = xpos_b[1:, 1].astype(np.float64)
    inv = 1.0 / (100.0 ** (np.arange(0, 32, 2, dtype=np.float64) / 32.0))
    angy = inv[:, None] * py[None, :]
    angx = inv[:, None] * px[None, :]
    c64 = np.concatenate([np.cos(angy), np.cos(angy), np.cos(angx), np.cos(angx)], 0)
    s64 = np.concatenate([np.sin(angy), np.sin(angy), np.sin(angx), np.sin(angx)], 0)
    c128 = np.concatenate([c64, c64], 0)
    s128 = np.concatenate([s64, s64], 0)
    bf = ml_dtypes.bfloat16
    return (np.ascontiguousarray(c128.astype(bf)),
            np.ascontiguousarray(s128.astype(bf)))


def _pmat2():
    P = np.zeros((64, 64), np.float32)
    for i in range(16):
        P[i, i + 16] = -1.0
        P[i + 16, i] = 1.0
        P[i + 32, i + 48] = -1.0
        P[i + 48, i + 32] = 1.0
    P2 = np.zeros((128, 128), np.float32)
    P2[:64, :64] = P
    P2[64:, 64:] = P
    return np.ascontiguousarray(P2.T.astype(ml_dtypes.bfloat16))


def kernel(**inputs):
    bf = ml_dtypes.bfloat16
    x = np.asarray(inputs["x"], np.float32)            # [8,1025,768]
    xpos = np.asarray(inputs["xpos"])                  # [8,1025,2]
    w_qkv = np.asarray(inputs["w_qkv"], np.float32)
    w_proj = np.asarray(inputs["w_proj"], np.float32)
    b_proj = np.asarray(inputs["b_proj"], np.float32).reshape(DIM)
    num_cls = int(np.asarray(inputs["num_cls"]))
    assert num_cls == 1, f"kernel specialized for num_cls=1, got {num_cls}"

    if "nc" not in _CACHE:
        _CACHE["nc"] = _build()
    nc = _CACHE["nc"]

    # shared (batch-independent) host tensors
    wqk = np.ascontiguousarray(w_qkv[0:1536].T.astype(bf))          # [768,1536]
    wv_t = w_qkv[1536:2304].T                                        # [768(in),768]
    wvp = np.zeros((DIM, 792), np.float32)
    for h in range(12):
        wvp[:, h * 66:h * 66 + 64] = wv_t[:, h * 64:(h + 1) * 64]
    wvp = np.ascontiguousarray(wvp.astype(bf))
    wp = np.ascontiguousarray(w_proj.T.astype(bf))                   # [768,768]
    pm2 = _pmat2()
    ident = np.ascontiguousarray(np.eye(128, dtype=np.float32))
    identb_h = np.ascontiguousarray(np.eye(128, dtype=np.float32).astype(bf))
    sel2 = np.zeros((2, 128), np.float32)
    sel2[0, 0:64] = 1.0
    sel2[1, 64:128] = 1.0
    sel2 = np.ascontiguousarray(sel2.astype(bf))
    selab_h = np.zeros((2, 97, 128), np.float32)
    selab_h[0, 0, 0:64] = 1.0    # qh0: head A rows
    selab_h[0, 32, 64:128] = 1.0
    selab_h[1, 64, 0:64] = 1.0   # qh1
    selab_h[1, 96, 64:128] = 1.0
    selab_h = np.ascontiguousarray(selab_h.astype(bf))
    ones12 = np.ones((128, 12), bf)
    biac = np.ascontiguousarray(b_proj.reshape(6, 128).T.astype(np.float32))
    bprow = np.ascontiguousarray(b_proj.reshape(1, DIM).astype(np.float32))

    in_maps = []
    for b in range(NC):
        c128, s128 = _host_tables(xpos[b])
        xt = np.ascontiguousarray(x[b, 1:, :].T.astype(bf))          # [768,1024]
        # host cls projections (f32)
        qkv_cls = w_qkv @ x[b, 0]                                    # [2304]
        q_cls, k_cls, v_cls = qkv_cls[0:768], qkv_cls[768:1536], qkv_cls[1536:2304]
        kcb = np.zeros((128, 12), np.float32)
        qcb = np.zeros((128, 12), np.float32)
        for h in range(12):
            hp = (h % 2) * 64
            kcb[hp:hp + 64, h] = k_cls[h * 64:(h + 1) * 64]
            qcb[hp:hp + 64, h] = q_cls[h * 64:(h + 1) * 64]
        vcr = np.zeros((1, 792), np.float32)
        vcm = np.zeros((2, 792), np.float32)
        for h in range(12):
            vcr[0, h * 66:h * 66 + 64] = v_cls[h * 64:(h + 1) * 64]
            vcr[0, h * 66 + 64] = 1.0
            vcm[h % 2, h * 66:h * 66 + 64] = v_cls[h * 64:(h + 1) * 64]
            vcm[h % 2, h * 66 + 64] = 1.0
        pcc = np.exp(SCALE * (q_cls.reshape(12, 64) * k_cls.reshape(12, 64)).sum(1))
        in_maps.append({
            "xt": xt, "wqk": wqk, "wv": wvp, "wp": wp,
            "ct": c128, "st": s128, "pm": pm2, "ident": ident,
            "kcb": np.ascontiguousarray(kcb.astype(bf)),
            "qcb": np.ascontiguousarray(qcb.astype(bf)),
            "vcm": np.ascontiguousarray(vcm.astype(bf)),
            "vcr": np.ascontiguousarray(vcr.astype(bf)),
            "identb": identb_h, "sel2": sel2, "selab": selab_h, "ones12": ones12,
            "pcc": np.ascontiguousarray(pcc.reshape(12, 1).astype(bf)),
            "biac": biac, "bprow": bprow,
        })
    res = run_bass_kernel_spmd(nc, in_maps, core_ids=list(range(NC)),
                               trace=bool(int(__import__("os").environ.get("BASS_TRACE_KERNEL", "0"))))
    _CACHE["last_result"] = res
    out = np.empty((NC, N, DIM), np.float32)
    for b, r in enumerate(res.results):
        out[b, 1:N, :] = r["outT"].T
        out[b, 0, :] = r["outc"][0]
    return out
